# revision 16
# baseline (speedup 1.0000x reference)
"""Causal MHA (B=2, S=2048, D=2048, H=16) on 8 trn2 NeuronCores.

Sharding: tensor-parallel over heads. Each core computes QKV + RoPE + causal
SDPA for H/8 heads end-to-end, then an AllToAll redistributes attention
outputs from head-sharded to token-sharded layout, and each core computes the
full out-projection for its 1/8 token slice.

The QKV projection and the attention are fused into ONE instruction stream:
after token-tile tt finishes (QKV+RoPE), the attention block for that
(batch, q-block) is sprinkled between the next tile's matmul steps, so the
Exp stream on the Scalar engine and the DVE softmax bookkeeping hide behind
the projection matmuls instead of serializing after them.

Layouts (partition dim = 128):
  xT      [D, T]  fp16, x transposed (T = B*S tokens, b-major)
  q/k     [128, HPC*T] fp16; per-head feature rows permuted [even;odd] so
          RoPE's pair rotation becomes a partition swap, folded into
          partition-shifted Activation copies (swap(z)*s == swap(z*swap(s)))
  v       [128, HPC*T] fp16 token-major per head
  scores  S^T tiles [tk=128, tq=512] f32 in PSUM; exp -> P fp16; causal
          masking via multiplicative 0/1 fp16 masks on DVE; softmax
          denominator accumulated on DVE in fp16 + one ones-matmul per block
  out-proj: w_out fp16 streamed during phase 3 on the GpSimd queue;
          two-pass PSUM accumulation (head-0 features first) hides the
          second head's AllToAll.
"""

import numpy as np

import concourse.bass as bass
import concourse.bacc as bacc
import concourse.mybir as mybir
import concourse.tile as tile
from concourse import bass_utils

F32 = mybir.dt.float32
F32R = mybir.dt.float32r
F16 = mybir.dt.float16


class Cfg:
    def __init__(self, B, S, D, H, NC=8):
        self.B, self.S, self.D, self.H, self.NC = B, S, D, H, NC
        self.DK = D // H
        assert self.DK == 128, "kernel assumes head dim 128"
        self.T = B * S                 # tokens, b-major
        self.HPC = H // NC             # heads per core
        self.FPC = self.HPC * self.DK  # features per core (q or k or v)
        self.W3 = 3 * self.FPC
        self.DCH = D // 128            # contraction chunks
        self.TT = 512                  # qkv token tile
        self.NTT = self.T // self.TT
        self.TQ = 512                  # attention tq tile
        self.SQT = S // self.TQ        # tq tiles per batch
        self.TPC = self.T // NC        # tokens per core for out-proj
        self.NTI = self.TPC // 128     # out-proj token chunks per core
        self.NJS = D // 512            # out-proj j tiles (512 wide)
        self.SCALE = float(1.0 / np.sqrt(self.DK))


FULL = Cfg(B=2, S=2048, D=2048, H=16, NC=8)


# --------------------------------------------------------------------------
# host-side prep
# --------------------------------------------------------------------------

def host_prep(cfg, x, w_qkv, w_out, cos, sin):
    B, S, D, H, NC = cfg.B, cfg.S, cfg.D, cfg.H, cfg.NC
    DK, T, HPC, FPC = cfg.DK, cfg.T, cfg.HPC, cfg.FPC

    x = np.asarray(x, dtype=np.float32)
    w_qkv = np.asarray(w_qkv, dtype=np.float32)
    w_out = np.asarray(w_out, dtype=np.float32)
    cos = np.asarray(cos, dtype=np.float32)
    sin = np.asarray(sin, dtype=np.float32)

    xT = np.ascontiguousarray(x.reshape(T, D).T.astype(np.float16))  # [D, T]

    # per-head [even; odd] feature permutation for q/k
    perm = np.concatenate([np.arange(0, DK, 2), np.arange(1, DK, 2)])
    w_q, w_k, w_v = w_qkv[0:D], w_qkv[D:2 * D], w_qkv[2 * D:3 * D]

    wqkvT = []
    for c in range(NC):
        rows = slice(FPC * c, FPC * (c + 1))
        wq = w_q[rows].reshape(HPC, DK, D)[:, perm, :].reshape(FPC, D)
        wk = w_k[rows].reshape(HPC, DK, D)[:, perm, :].reshape(FPC, D)
        wv = w_v[rows]
        wqkvT.append(np.ascontiguousarray(
            np.concatenate([wq, wk, wv], axis=0).T.astype(np.float16)))

    cosT = np.tile(cos.T, (1, B))                                  # [64, T]
    sinT = np.tile(sin.T, (1, B))
    cosF = np.ascontiguousarray(np.concatenate([cosT, cosT], axis=0), dtype=np.float32)
    # pre-swapped sin so rope's swap folds into partition-shifted copies:
    # swap(z)*[-s;s] == swap(z*[s;-s])
    sinF = np.ascontiguousarray(np.concatenate([sinT, -sinT], axis=0), dtype=np.float32)

    # multiplicative causal masks for S^T diagonal tiles [128, 4*TQ], fp16
    i = np.arange(128)[:, None]
    j = np.arange(cfg.TQ)[None, :]
    masks = np.concatenate(
        [np.where(i <= j - 128 * m, 1.0, 0.0).astype(np.float16) for m in range(4)],
        axis=1,
    )
    masks = np.ascontiguousarray(masks)

    woutT = np.ascontiguousarray(w_out.T.astype(np.float16))       # [D(f), D(j)]

    ones = np.ones((128, 128), dtype=np.float16)
    shared = dict(xT=xT, cosF=cosF, sinF=sinF, masks=masks, ones=ones, woutT=woutT)
    return shared, wqkvT


# --------------------------------------------------------------------------
# device program
# --------------------------------------------------------------------------

def build_program(cfg):
    nc = bacc.Bacc(
        "TRN2",
        target_bir_lowering=False,
        debug=False,
        num_devices=cfg.NC,
    )

    xT_d = nc.dram_tensor("xT", [cfg.D, cfg.T], F16, kind="ExternalInput").ap()
    wqkvT_d = nc.dram_tensor("wqkvT", [cfg.D, cfg.W3], F16, kind="ExternalInput").ap()
    cosF_d = nc.dram_tensor("cosF", [128, cfg.T], F32, kind="ExternalInput").ap()
    sinF_d = nc.dram_tensor("sinF", [128, cfg.T], F32, kind="ExternalInput").ap()
    masks_d = nc.dram_tensor("masks", [128, 4 * cfg.TQ], F16, kind="ExternalInput").ap()
    ones_d = nc.dram_tensor("ones", [128, 128], F16, kind="ExternalInput").ap()
    woutT_d = nc.dram_tensor("woutT", [cfg.D, cfg.D], F16, kind="ExternalInput").ap()
    out_d = nc.dram_tensor("out", [cfg.TPC, cfg.D], F32, kind="ExternalOutput").ap()

    with tile.TileContext(nc) as tc:
        _build_body(
            nc, tc, cfg,
            xT_d, wqkvT_d, cosF_d, sinF_d, masks_d, ones_d, woutT_d, out_d,
        )

    nc.compile()
    return nc


def _stream(nc, tc, cfg, xT_d, wqkvT_d, cosF_d, sinF_d, q_sb, k_sb, v_sb,
            oT_sb, masks_sb, ones_sb):
    """Fused QKV+RoPE+attention stream."""
    T, S, HPC, FPC, W3 = cfg.T, cfg.S, cfg.HPC, cfg.FPC, cfg.W3
    DCH, NTT, TT, TQ = cfg.DCH, cfg.NTT, cfg.TT, cfg.TQ

    with (
        tc.tile_pool(name="wqkv", bufs=1) as wp,
        tc.tile_pool(name="xin", bufs=24) as xp,
        tc.tile_pool(name="csin", bufs=2) as csp,
        tc.tile_pool(name="ropet", bufs=2) as rtp,
        tc.tile_pool(name="pexp", bufs=6) as pep,
        tc.tile_pool(name="lacc", bufs=3) as lap,
        tc.tile_pool(name="linv", bufs=2) as lip,
        tc.tile_pool(name="pqk", bufs=2, space="PSUM") as pqkp,
        tc.tile_pool(name="pv", bufs=2, space="PSUM") as pvp,
        tc.tile_pool(name="pss", bufs=2, space="PSUM") as pssp,
        tc.tile_pool(name="pso", bufs=1, space="PSUM") as psop,
        tc.tile_pool(name="psl", bufs=1, space="PSUM") as pslp,
    ):
        w_ts = []
        for dc in range(DCH):
            w_t = wp.tile([128, W3], F16, name=f"w_{dc}")
            nc.scalar.dma_start(w_t[:], wqkvT_d[128 * dc:128 * (dc + 1), :])
            w_ts.append(w_t)

        onesr = ones_sb[:]

        # ---------- attention block closures ----------
        def block_closures(hc, b, jq):
            base = hc * T + S * b
            nkc = 4 * (jq + 1)
            ctx = {}

            def fst():
                ctx["o"] = psop.tile([128, TQ], F32, tag="o",
                                     name=f"o_{hc}_{b}_{jq}")
                ctx["la"] = lap.tile([128, TQ], F16, tag="la",
                                     name=f"la_{hc}_{b}_{jq}")
                ctx["p"] = {}

            def mk_s(ck):
                def f():
                    s_ps = pssp.tile([128, TQ], F32, tag="s",
                                     name=f"s_{hc}_{b}_{jq}_{ck}")
                    ksl = k_sb[:, base + 128 * ck:base + 128 * (ck + 1)]
                    qsl = q_sb[:, base + TQ * jq:base + TQ * (jq + 1)]
                    nc.tensor.matmul(s_ps[:], ksl, qsl, start=True, stop=True)
                    p_sb = pep.tile([128, TQ], F16, tag="p",
                                    name=f"p_{hc}_{b}_{jq}_{ck}")
                    nc.scalar.activation(
                        p_sb[:], s_ps[:], mybir.ActivationFunctionType.Exp,
                        scale=cfg.SCALE,
                    )
                    m = ck - 4 * jq
                    if m >= 0:
                        nc.vector.tensor_mul(
                            p_sb[:], p_sb[:], masks_sb[:, TQ * m:TQ * (m + 1)]
                        )
                    if ck == 0:
                        nc.vector.tensor_copy(ctx["la"][:], p_sb[:])
                    else:
                        nc.vector.tensor_add(ctx["la"][:], ctx["la"][:], p_sb[:])
                    ctx["p"][ck] = p_sb
                return f

            def mk_pv(ck):
                def f():
                    p_sb = ctx["p"].pop(ck)
                    g = (S // 128) * b + ck
                    vsl = v_sb[:, hc * T + 128 * g:hc * T + 128 * (g + 1)]
                    nc.tensor.matmul(
                        ctx["o"][:], vsl, p_sb[:],
                        start=(ck == 0), stop=(ck == nkc - 1),
                    )
                return f

            def tail():
                l_ps = pslp.tile([128, TQ], F32, tag="l",
                                 name=f"l_{hc}_{b}_{jq}")
                nc.tensor.matmul(l_ps[:], onesr, ctx["la"][:],
                                 start=True, stop=True)
                linv = lip.tile([128, TQ], F32, tag="li",
                                name=f"li_{hc}_{b}_{jq}")
                nc.vector.reciprocal_approx_fast(linv[:], l_ps[:])
                nc.vector.tensor_mul(
                    oT_sb[:, base + TQ * jq:base + TQ * (jq + 1)],
                    ctx["o"][:], linv[:],
                )

            # schedule: s(ck) with pv lagging 2 so the Exp hides behind
            # interleaved projection matmuls
            ops = []
            s_fns = [mk_s(ck) for ck in range(nkc)]
            pv_fns = [mk_pv(ck) for ck in range(nkc)]

            def chain(fs):
                def f():
                    for g in fs:
                        g()
                return f

            ops.append(chain([fst, s_fns[0]]))
            if nkc > 1:
                ops.append(s_fns[1])
            for ck in range(2, nkc):
                ops.append(chain([pv_fns[ck - 2], s_fns[ck]]))
            ops.append(pv_fns[nkc - 2] if nkc >= 2 else chain([]))
            ops.append(chain([pv_fns[nkc - 1], tail]))
            return ops

        # ---------- fused stream ----------
        pending = []          # closures of the previous tile's block

        for tt in range(NTT):
            b_cur, jql = tt // 4, tt % 4

            xts = []
            x_t0 = xp.tile([128, TT], F16, tag="x", name=f"x_{tt}_0")
            nc.sync.dma_start(x_t0[:], xT_d[0:128, TT * tt:TT * (tt + 1)])
            xts.append(x_t0)
            cos_t = csp.tile([128, TT], F32, tag="cos")
            nc.sync.dma_start(cos_t[:], cosF_d[:, TT * tt:TT * (tt + 1)])
            sin_t = csp.tile([128, TT], F32, tag="sin")
            nc.sync.dma_start(sin_t[:], sinF_d[:, TT * tt:TT * (tt + 1)])
            for dc in range(1, DCH):
                x_t = xp.tile([128, TT], F16, tag="x", name=f"x_{tt}_{dc}")
                nc.sync.dma_start(
                    x_t[:], xT_d[128 * dc:128 * (dc + 1), TT * tt:TT * (tt + 1)]
                )
                xts.append(x_t)

            # sprinkle plan: pending ops over the 2*DCH dc-steps, skipping
            # the first few steps so the previous tile's rope can drain
            nsteps = 2 * DCH
            skip = 4 if tt > 0 else nsteps
            L = len(pending)
            spr = {}
            if L:
                usable = nsteps - skip
                for i in range(usable):
                    lo = (i * L) // usable
                    hi = ((i + 1) * L) // usable
                    if hi > lo:
                        spr[skip + i] = pending[lo:hi]

            pvt = [pvp.tile([128, 2 * FPC], F32, tag="v", name=f"pv_{tt}_{i}")
                   for i in range(2)]
            pvs = [pvt[ci // 2][:, FPC * (ci % 2):FPC * (ci % 2 + 1)]
                   for ci in range(4)]

            def rope(oc):
                dst = q_sb if oc < HPC else k_sb
                hc = oc % HPC
                z = ctx_pq[oc]
                d0 = hc * T + TT * tt
                t1 = rtp.tile([128, TT], F32, tag="t1", name=f"t1_{tt}_{oc}")
                u = rtp.tile([128, TT], F32, tag="u", name=f"u_{tt}_{oc}")
                usw = rtp.tile([128, TT], F32, tag="usw", name=f"usw_{tt}_{oc}")
                nc.vector.tensor_mul(t1[:], z[:], cos_t[:])
                nc.vector.tensor_mul(u[:], z[:], sin_t[:])
                nc.scalar.copy(usw[0:64, :], u[64:128, :])
                nc.scalar.copy(usw[64:128, :], u[0:64, :])
                nc.vector.tensor_add(dst[:, d0:d0 + TT], t1[:], usw[:])

            ctx_pq = {}
            step = 0
            for pss in range(2):          # pass 0: q + v(0,1); pass 1: k + v(2,3)
                ocs = (0, 1) if pss == 0 else (2, 3)
                cis = (0, 1) if pss == 0 else (2, 3)
                for oc in ocs:
                    ctx_pq[oc] = pqkp.tile([128, TT], F32, tag="qk",
                                           name=f"pq_{tt}_{oc}")
                for dc in range(DCH):
                    xr = xts[dc][:]
                    first, last = dc == 0, dc == DCH - 1
                    for oc in ocs:
                        wsl = w_ts[dc][:, 128 * oc:128 * (oc + 1)]
                        nc.tensor.matmul(
                            ctx_pq[oc][:], wsl, xr, start=first, stop=last
                        )
                    wv = w_ts[dc][:, 2 * FPC:W3]
                    for ci in cis:
                        lhs = xts[dc][:, 128 * ci:128 * (ci + 1)]
                        nc.tensor.matmul(
                            pvs[ci][:], lhs, wv,
                            start=(first and ci % 2 == 0), stop=last,
                            skip_group_check=(ci % 2 == 1),
                        )
                    for f in spr.get(step, ()):
                        f()
                    step += 1
                for oc in ocs:
                    rope(oc)
                if pss == 1:
                    for ci in range(4):
                        g = 4 * tt + ci
                        for hc in range(HPC):
                            nc.scalar.copy(
                                v_sb[:, hc * T + 128 * g:hc * T + 128 * (g + 1)],
                                pvs[ci][:, 128 * hc:128 * (hc + 1)],
                            )

            pending = (block_closures(0, b_cur, jql)
                       + block_closures(1, b_cur, jql))

        # final block (b=1, jq=3): no projection work left to hide behind
        for f in pending:
            f()


def _phase3_outproj(nc, tc, cfg, oT_sb, woutT_d, out_d):
    T, S, HPC, DCH = cfg.T, cfg.S, cfg.HPC, cfg.DCH

    with (
        tc.tile_pool(name="dram", bufs=1, space="DRAM") as dramp,
        tc.tile_pool(name="rhsp", bufs=1) as rhsp,
        tc.tile_pool(name="woutp", bufs=2) as woutp,
        tc.tile_pool(name="osb", bufs=3) as osbp,
        tc.tile_pool(name="pout", bufs=8, space="PSUM") as poutp,
    ):
        # One A2A per head; head 0's fires while head 1's attention tail is
        # still running (collectives run off the 5 engines).
        a2a_outs = []
        for hc in range(HPC):
            ob = dramp.tile([cfg.NC * 128, cfg.TPC], F16, name=f"obounce{hc}")
            for s in range(cfg.NC):
                t0 = cfg.TPC * s
                b, sb0 = t0 // S, t0 % S
                nc.sync.dma_start(
                    ob[128 * s:128 * (s + 1), :],
                    oT_sb[:, hc * T + S * b + sb0:
                          hc * T + S * b + sb0 + cfg.TPC],
                )
            ao = dramp.tile([cfg.NC * 128, cfg.TPC], F16, name=f"a2a_out{hc}")
            nc.gpsimd.collective_compute(
                "AllToAll",
                mybir.AluOpType.bypass,
                replica_groups=[list(range(cfg.NC))],
                ins=[ob[:].opt()],
                outs=[ao[:].opt()],
            )
            a2a_outs.append(ao)

        # stream w_out per j-slice on the gpsimd queue (idle in phase 3)
        wout_js = []
        for js in range(cfg.NJS):
            wt = woutp.tile([128, DCH * 512], F16, tag="wj", name=f"wj_{js}")
            for fc in range(DCH):
                nc.gpsimd.dma_start(
                    wt[:, 512 * fc:512 * (fc + 1)],
                    woutT_d[128 * fc:128 * (fc + 1), 512 * js:512 * (js + 1)],
                )
            wout_js.append(wt)

        # gather each feature chunk; even fc come from head 0 (early A2A)
        rhs_scr = rhsp.tile([128, DCH * cfg.TPC], F16, name="rhs_scr")
        rhs = []
        for fc in range(DCH):
            r_, hc = fc // HPC, fc % HPC
            sl = rhs_scr[:, cfg.TPC * fc:cfg.TPC * (fc + 1)]
            nc.sync.dma_start(sl, a2a_outs[hc][128 * r_:128 * (r_ + 1), :])
            rhs.append(sl)

        fcs_a = [fc for fc in range(DCH) if fc % HPC == 0]   # head 0 features
        fcs_b = [fc for fc in range(DCH) if fc % HPC != 0]   # head 1 features

        # two-pass accumulation over js-pairs: pass A (head-0 features) can
        # start as soon as the first A2A lands, hiding the second A2A
        for jp in range(cfg.NJS // 2):
            pss = {}
            for js in (2 * jp, 2 * jp + 1):
                for ti in range(cfg.NTI):
                    ps = poutp.tile([128, 512], F32, tag="po",
                                    name=f"po_{js}_{ti}")
                    pss[(js, ti)] = ps
                    for i, fc in enumerate(fcs_a):
                        nc.tensor.matmul(
                            ps[:],
                            rhs[fc][:, 128 * ti:128 * (ti + 1)],
                            wout_js[js][:, 512 * fc:512 * (fc + 1)],
                            start=(i == 0), stop=False,
                        )
            for js in (2 * jp, 2 * jp + 1):
                for ti in range(cfg.NTI):
                    ps = pss[(js, ti)]
                    for i, fc in enumerate(fcs_b):
                        nc.tensor.matmul(
                            ps[:],
                            rhs[fc][:, 128 * ti:128 * (ti + 1)],
                            wout_js[js][:, 512 * fc:512 * (fc + 1)],
                            start=False, stop=(i == len(fcs_b) - 1),
                        )
                    osb = osbp.tile([128, 512], F32, tag="ob",
                                    name=f"ob_{js}_{ti}")
                    nc.vector.tensor_copy(osb[:], ps[:])
                    nc.sync.dma_start(
                        out_d[128 * ti:128 * (ti + 1), 512 * js:512 * (js + 1)],
                        osb[:],
                    )


def _build_body(nc, tc, cfg, xT_d, wqkvT_d, cosF_d, sinF_d, masks_d, ones_d,
                woutT_d, out_d):
    T, HPC, TQ = cfg.T, cfg.HPC, cfg.TQ

    with tc.tile_pool(name="const", bufs=1) as constp:
        ones_sb = constp.tile([128, 128], F16)
        nc.gpsimd.dma_start(ones_sb[:], ones_d[:])
        masks_sb = constp.tile([128, 4 * TQ], F16)
        nc.gpsimd.dma_start(masks_sb[:], masks_d[:])
        # warm the Exp activation table off the critical path
        warm = constp.tile([128, 1], F32, name="actwarm")
        nc.scalar.activation(
            warm[:], ones_sb[:, 0:1], mybir.ActivationFunctionType.Exp
        )

        with tc.tile_pool(name="qkvp", bufs=1) as qkvp:
            q_sb = qkvp.tile([128, HPC * T], F16)
            k_sb = qkvp.tile([128, HPC * T], F16)
            v_sb = qkvp.tile([128, HPC * T], F16)
            oT_sb = qkvp.tile([128, HPC * T], F16)

            _stream(nc, tc, cfg, xT_d, wqkvT_d, cosF_d, sinF_d,
                    q_sb, k_sb, v_sb, oT_sb, masks_sb, ones_sb)
            _phase3_outproj(nc, tc, cfg, oT_sb, woutT_d, out_d)


# --------------------------------------------------------------------------
# host entry point
# --------------------------------------------------------------------------

_CACHE = {}


def _compiled(cfg):
    key = (cfg.B, cfg.S, cfg.D, cfg.H, cfg.NC)
    if key not in _CACHE:
        _CACHE[key] = build_program(cfg)
    return _CACHE[key]


def make_in_maps(cfg, inputs):
    shared, wqkvT = host_prep(
        cfg, inputs["x"], inputs["w_qkv"], inputs["w_out"],
        inputs["cos"], inputs["sin"],
    )
    return [{**shared, "wqkvT": wqkvT[c]} for c in range(cfg.NC)]


def assemble(cfg, results):
    out = np.concatenate([results[c]["out"] for c in range(cfg.NC)], axis=0)
    return out.reshape(cfg.B, cfg.S, cfg.D).astype(np.float32)


def kernel(x, w_qkv, w_out, cos, sin):
    cfg = FULL
    nc = _compiled(cfg)
    in_maps = make_in_maps(cfg, dict(x=x, w_qkv=w_qkv, w_out=w_out, cos=cos, sin=sin))
    res = bass_utils.run_bass_kernel_spmd(nc, in_maps, core_ids=list(range(cfg.NC)))
    return assemble(cfg, res.results)


# revision 25
# speedup vs baseline: 1.0317x; 1.0317x over previous
"""Causal MHA (B=2, S=2048, D=2048, H=16) on 8 trn2 NeuronCores.

Sharding: tensor-parallel over heads. Each core computes QKV + RoPE + causal
SDPA for H/8 heads end-to-end, then an AllToAll redistributes attention
outputs from head-sharded to token-sharded layout, and each core computes the
full out-projection for its 1/8 token slice.

The QKV projection and the attention are fused into ONE instruction stream:
after token-tile tt finishes (QKV+RoPE), the attention block for that
(batch, q-block) is sprinkled between the next tile's matmul steps, so the
Exp stream on the Scalar engine and the DVE softmax bookkeeping hide behind
the projection matmuls instead of serializing after them.

Layouts (partition dim = 128):
  xT      [D, T]  fp16, x transposed (T = B*S tokens, b-major)
  q/k     [128, HPC*T] fp16; per-head feature rows permuted [even;odd] so
          RoPE's pair rotation becomes a partition swap, folded into
          partition-shifted Activation copies (swap(z)*s == swap(z*swap(s)))
  v       [128, HPC*T] fp16 token-major per head
  scores  S^T tiles [tk=128, tq=512] f32 in PSUM; exp -> P fp16; causal
          masking via multiplicative 0/1 fp16 masks on DVE; softmax
          denominator accumulated on DVE in fp16 + one ones-matmul per block
  out-proj: w_out fp16 streamed during phase 3 on the GpSimd queue;
          two-pass PSUM accumulation (head-0 features first) hides the
          second head's AllToAll.
"""

import numpy as np

import concourse.bass as bass
import concourse.bacc as bacc
import concourse.mybir as mybir
import concourse.tile as tile
from concourse import bass_utils

F32 = mybir.dt.float32
F32R = mybir.dt.float32r
F16 = mybir.dt.float16


class Cfg:
    def __init__(self, B, S, D, H, NC=8):
        self.B, self.S, self.D, self.H, self.NC = B, S, D, H, NC
        self.DK = D // H
        assert self.DK == 128, "kernel assumes head dim 128"
        self.T = B * S                 # tokens, b-major
        self.HPC = H // NC             # heads per core
        self.FPC = self.HPC * self.DK  # features per core (q or k or v)
        self.W3 = 3 * self.FPC
        self.DCH = D // 128            # contraction chunks
        self.TT = 512                  # qkv token tile
        self.NTT = self.T // self.TT
        self.TQ = 512                  # attention tq tile
        self.SQT = S // self.TQ        # tq tiles per batch
        self.TPC = self.T // NC        # tokens per core for out-proj
        self.NTI = self.TPC // 128     # out-proj token chunks per core
        self.NJS = D // 512            # out-proj j tiles (512 wide)
        self.SCALE = float(1.0 / np.sqrt(self.DK))


FULL = Cfg(B=2, S=2048, D=2048, H=16, NC=8)


# --------------------------------------------------------------------------
# host-side prep
# --------------------------------------------------------------------------

def host_prep(cfg, x, w_qkv, w_out, cos, sin):
    B, S, D, H, NC = cfg.B, cfg.S, cfg.D, cfg.H, cfg.NC
    DK, T, HPC, FPC = cfg.DK, cfg.T, cfg.HPC, cfg.FPC

    x = np.asarray(x, dtype=np.float32)
    w_qkv = np.asarray(w_qkv, dtype=np.float32)
    w_out = np.asarray(w_out, dtype=np.float32)
    cos = np.asarray(cos, dtype=np.float32)
    sin = np.asarray(sin, dtype=np.float32)

    # xT rearranged so each token tile tt is ONE contiguous [128, DCH*TT]
    # DMA: xP[p, (tt*DCH + dc)*TT + c] = xT[128*dc + p, TT*tt + c]
    xT = x.reshape(T, D).T.astype(np.float16)                      # [D, T]
    NTT, DCH, TT = cfg.NTT, cfg.DCH, cfg.TT
    xP = np.ascontiguousarray(
        xT.reshape(DCH, 128, NTT, TT).transpose(1, 2, 0, 3).reshape(128, T * DCH)
    )

    # per-head [even; odd] feature permutation for q/k
    perm = np.concatenate([np.arange(0, DK, 2), np.arange(1, DK, 2)])
    w_q, w_k, w_v = w_qkv[0:D], w_qkv[D:2 * D], w_qkv[2 * D:3 * D]

    wqkvT = []
    for c in range(NC):
        rows = slice(FPC * c, FPC * (c + 1))
        wq = w_q[rows].reshape(HPC, DK, D)[:, perm, :].reshape(FPC, D)
        wk = w_k[rows].reshape(HPC, DK, D)[:, perm, :].reshape(FPC, D)
        wv = w_v[rows]
        wqkvT.append(np.ascontiguousarray(
            np.concatenate([wq, wk, wv], axis=0).T.astype(np.float16)))

    cosT = np.tile(cos.T, (1, B))                                  # [64, T]
    sinT = np.tile(sin.T, (1, B))
    cosF = np.ascontiguousarray(np.concatenate([cosT, cosT], axis=0), dtype=np.float32)
    # pre-swapped sin so rope's swap folds into partition-shifted copies:
    # swap(z)*[-s;s] == swap(z*[s;-s])
    sinF = np.ascontiguousarray(np.concatenate([sinT, -sinT], axis=0), dtype=np.float32)

    # multiplicative causal masks for S^T diagonal tiles [128, 4*TQ], fp16
    i = np.arange(128)[:, None]
    j = np.arange(cfg.TQ)[None, :]
    masks = np.concatenate(
        [np.where(i <= j - 128 * m, 1.0, 0.0).astype(np.float16) for m in range(4)],
        axis=1,
    )
    masks = np.ascontiguousarray(masks)

    # w_out^T rearranged so each 512-wide j-slice is ONE contiguous DMA:
    # woutP[p, (js*DCH + fc)*512 + c] = woutT[128*fc + p, 512*js + c]
    woutT = w_out.T.astype(np.float16)                             # [D(f), D(j)]
    NJS = cfg.NJS
    woutP = np.ascontiguousarray(
        woutT.reshape(DCH, 128, NJS, 512).transpose(1, 2, 0, 3).reshape(128, D * DCH)
    )

    ones = np.ones((128, 128), dtype=np.float16)
    shared = dict(xT=xP, cosF=cosF, sinF=sinF, masks=masks, ones=ones, woutT=woutP)
    return shared, wqkvT


# --------------------------------------------------------------------------
# device program
# --------------------------------------------------------------------------

def build_program(cfg):
    nc = bacc.Bacc(
        "TRN2",
        target_bir_lowering=False,
        debug=False,
        num_devices=cfg.NC,
    )

    xT_d = nc.dram_tensor("xT", [128, cfg.T * cfg.DCH], F16, kind="ExternalInput").ap()
    wqkvT_d = nc.dram_tensor("wqkvT", [cfg.D, cfg.W3], F16, kind="ExternalInput").ap()
    cosF_d = nc.dram_tensor("cosF", [128, cfg.T], F32, kind="ExternalInput").ap()
    sinF_d = nc.dram_tensor("sinF", [128, cfg.T], F32, kind="ExternalInput").ap()
    masks_d = nc.dram_tensor("masks", [128, 4 * cfg.TQ], F16, kind="ExternalInput").ap()
    ones_d = nc.dram_tensor("ones", [128, 128], F16, kind="ExternalInput").ap()
    woutT_d = nc.dram_tensor("woutT", [128, cfg.D * cfg.DCH], F16, kind="ExternalInput").ap()
    out_d = nc.dram_tensor("out", [cfg.TPC, cfg.D], F32, kind="ExternalOutput").ap()

    with tile.TileContext(nc) as tc:
        _build_body(
            nc, tc, cfg,
            xT_d, wqkvT_d, cosF_d, sinF_d, masks_d, ones_d, woutT_d, out_d,
        )

    nc.compile()
    return nc


def _stream(nc, tc, cfg, xT_d, wqkvT_d, cosF_d, sinF_d, q_sb, k_sb, v_sb,
            oT_sb, masks_sb, ones_sb):
    """Fused QKV+RoPE+attention stream."""
    T, S, HPC, FPC, W3 = cfg.T, cfg.S, cfg.HPC, cfg.FPC, cfg.W3
    DCH, NTT, TT, TQ = cfg.DCH, cfg.NTT, cfg.TT, cfg.TQ

    with (
        tc.tile_pool(name="wqkv", bufs=1) as wp,
        tc.tile_pool(name="xin", bufs=2) as xp,
        tc.tile_pool(name="csin", bufs=2) as csp,
        tc.tile_pool(name="ropet", bufs=2) as rtp,
        tc.tile_pool(name="pexp", bufs=6) as pep,
        tc.tile_pool(name="lacc", bufs=3) as lap,
        tc.tile_pool(name="linv", bufs=2) as lip,
        tc.tile_pool(name="pqk", bufs=2, space="PSUM") as pqkp,
        tc.tile_pool(name="pv", bufs=2, space="PSUM") as pvp,
        tc.tile_pool(name="pss", bufs=2, space="PSUM") as pssp,
        tc.tile_pool(name="pso", bufs=1, space="PSUM") as psop,
        tc.tile_pool(name="psl", bufs=1, space="PSUM") as pslp,
    ):
        w_ts = []
        for dc in range(DCH):
            w_t = wp.tile([128, W3], F16, name=f"w_{dc}")
            nc.scalar.dma_start(w_t[:], wqkvT_d[128 * dc:128 * (dc + 1), :])
            w_ts.append(w_t)

        onesr = ones_sb[:]

        # ---------- attention block closures ----------
        def block_closures(hc, b, jq, pe_denom=False):
            # pe_denom: accumulate the softmax denominator on the PE (one
            # ones-matmul per key chunk) instead of DVE — used for the final
            # blocks so their completion doesn't wait on the DVE queue
            # (the AllToAll hangs off their omult).
            base = hc * T + S * b
            nkc = 4 * (jq + 1)
            ctx = {}

            def fst():
                ctx["o"] = psop.tile([128, TQ], F32, tag="o",
                                     name=f"o_{hc}_{b}_{jq}")
                if pe_denom:
                    ctx["l"] = pslp.tile([128, TQ], F32, tag="l",
                                         name=f"l_{hc}_{b}_{jq}")
                else:
                    ctx["la"] = lap.tile([128, TQ], F16, tag="la",
                                         name=f"la_{hc}_{b}_{jq}")
                ctx["p"] = {}

            def mk_s(ck):
                def f():
                    s_ps = pssp.tile([128, TQ], F32, tag="s",
                                     name=f"s_{hc}_{b}_{jq}_{ck}")
                    ksl = k_sb[:, base + 128 * ck:base + 128 * (ck + 1)]
                    qsl = q_sb[:, base + TQ * jq:base + TQ * (jq + 1)]
                    nc.tensor.matmul(s_ps[:], ksl, qsl, start=True, stop=True)
                    p_sb = pep.tile([128, TQ], F16, tag="p",
                                    name=f"p_{hc}_{b}_{jq}_{ck}")
                    nc.scalar.activation(
                        p_sb[:], s_ps[:], mybir.ActivationFunctionType.Exp,
                        scale=cfg.SCALE,
                    )
                    m = ck - 4 * jq
                    if m >= 0:
                        nc.vector.tensor_mul(
                            p_sb[:], p_sb[:], masks_sb[:, TQ * m:TQ * (m + 1)]
                        )
                    if not pe_denom:
                        if ck == 0:
                            nc.vector.tensor_copy(ctx["la"][:], p_sb[:])
                        else:
                            nc.vector.tensor_add(ctx["la"][:], ctx["la"][:],
                                                 p_sb[:])
                    ctx["p"][ck] = p_sb
                return f

            def mk_pv(ck):
                def f():
                    p_sb = ctx["p"].pop(ck)
                    g = (S // 128) * b + ck
                    vsl = v_sb[:, hc * T + 128 * g:hc * T + 128 * (g + 1)]
                    nc.tensor.matmul(
                        ctx["o"][:], vsl, p_sb[:],
                        start=(ck == 0), stop=(ck == nkc - 1),
                    )
                    if pe_denom:
                        nc.tensor.matmul(
                            ctx["l"][:], onesr, p_sb[:],
                            start=(ck == 0), stop=(ck == nkc - 1),
                        )
                return f

            def tail():
                if pe_denom:
                    l_ps = ctx["l"]
                else:
                    l_ps = pslp.tile([128, TQ], F32, tag="l",
                                     name=f"l_{hc}_{b}_{jq}")
                    nc.tensor.matmul(l_ps[:], onesr, ctx["la"][:],
                                     start=True, stop=True)
                linv = lip.tile([128, TQ], F32, tag="li",
                                name=f"li_{hc}_{b}_{jq}")
                nc.vector.reciprocal_approx_fast(linv[:], l_ps[:])
                nc.vector.tensor_mul(
                    oT_sb[:, base + TQ * jq:base + TQ * (jq + 1)],
                    ctx["o"][:], linv[:],
                )

            # schedule: s(ck) with pv lagging 2 so the Exp hides behind
            # interleaved projection matmuls
            ops = []
            s_fns = [mk_s(ck) for ck in range(nkc)]
            pv_fns = [mk_pv(ck) for ck in range(nkc)]

            def chain(fs):
                def f():
                    for g in fs:
                        g()
                return f

            ops.append(chain([fst, s_fns[0]]))
            if nkc > 1:
                ops.append(s_fns[1])
            for ck in range(2, nkc):
                ops.append(chain([pv_fns[ck - 2], s_fns[ck]]))
            ops.append(pv_fns[nkc - 2] if nkc >= 2 else chain([]))
            ops.append(chain([pv_fns[nkc - 1], tail]))
            return ops

        # ---------- fused stream ----------
        pending = []          # closures of the previous tile's block

        for tt in range(NTT):
            b_cur, jql = tt // 4, tt % 4

            # one contiguous DMA per token tile (host pre-arranged); tt=0 is
            # split per-dc so the first matmul starts as early as possible
            xfull = xp.tile([128, DCH * TT], F16, tag="x", name=f"x_{tt}")
            xbase = tt * DCH * TT
            if tt == 0:
                nc.sync.dma_start(xfull[:, 0:TT], xT_d[:, xbase:xbase + TT])
            cos_t = csp.tile([128, TT], F32, tag="cos")
            nc.sync.dma_start(cos_t[:], cosF_d[:, TT * tt:TT * (tt + 1)])
            sin_t = csp.tile([128, TT], F32, tag="sin")
            nc.sync.dma_start(sin_t[:], sinF_d[:, TT * tt:TT * (tt + 1)])
            if tt == 0:
                for dc in range(1, DCH):
                    nc.sync.dma_start(
                        xfull[:, TT * dc:TT * (dc + 1)],
                        xT_d[:, xbase + TT * dc:xbase + TT * (dc + 1)],
                    )
            else:
                nc.sync.dma_start(xfull[:], xT_d[:, xbase:xbase + DCH * TT])
            xts = [xfull[:, TT * dc:TT * (dc + 1)] for dc in range(DCH)]

            # sprinkle plan: pending ops over the 2*DCH dc-steps, skipping
            # the first few steps so the previous tile's rope can drain
            nsteps = 2 * DCH
            skip = 4 if tt > 0 else nsteps
            L = len(pending)
            spr = {}
            if L:
                usable = nsteps - skip
                for i in range(usable):
                    lo = (i * L) // usable
                    hi = ((i + 1) * L) // usable
                    if hi > lo:
                        spr[skip + i] = pending[lo:hi]

            pvt = [pvp.tile([128, 2 * FPC], F32, tag="v", name=f"pv_{tt}_{i}")
                   for i in range(2)]
            pvs = [pvt[ci // 2][:, FPC * (ci % 2):FPC * (ci % 2 + 1)]
                   for ci in range(4)]

            def rope(oc):
                dst = q_sb if oc < HPC else k_sb
                hc = oc % HPC
                z = ctx_pq[oc]
                d0 = hc * T + TT * tt
                t1 = rtp.tile([128, TT], F32, tag="t1", name=f"t1_{tt}_{oc}")
                u = rtp.tile([128, TT], F32, tag="u", name=f"u_{tt}_{oc}")
                usw = rtp.tile([128, TT], F32, tag="usw", name=f"usw_{tt}_{oc}")
                nc.vector.tensor_mul(t1[:], z[:], cos_t[:])
                nc.vector.tensor_mul(u[:], z[:], sin_t[:])
                nc.scalar.copy(usw[0:64, :], u[64:128, :])
                nc.scalar.copy(usw[64:128, :], u[0:64, :])
                nc.vector.tensor_add(dst[:, d0:d0 + TT], t1[:], usw[:])

            ctx_pq = {}
            step = 0
            for pss in range(2):          # pass 0: q + v(0,1); pass 1: k + v(2,3)
                ocs = (0, 1) if pss == 0 else (2, 3)
                cis = (0, 1) if pss == 0 else (2, 3)
                for oc in ocs:
                    ctx_pq[oc] = pqkp.tile([128, TT], F32, tag="qk",
                                           name=f"pq_{tt}_{oc}")
                for dc in range(DCH):
                    xr = xts[dc][:]
                    first, last = dc == 0, dc == DCH - 1
                    for oc in ocs:
                        wsl = w_ts[dc][:, 128 * oc:128 * (oc + 1)]
                        nc.tensor.matmul(
                            ctx_pq[oc][:], wsl, xr, start=first, stop=last
                        )
                    wv = w_ts[dc][:, 2 * FPC:W3]
                    for ci in cis:
                        lhs = xts[dc][:, 128 * ci:128 * (ci + 1)]
                        nc.tensor.matmul(
                            pvs[ci][:], lhs, wv,
                            start=(first and ci % 2 == 0), stop=last,
                            skip_group_check=(ci % 2 == 1),
                        )
                    for f in spr.get(step, ()):
                        f()
                    step += 1
                for oc in ocs:
                    rope(oc)
                if pss == 1:
                    for ci in range(4):
                        g = 4 * tt + ci
                        for hc in range(HPC):
                            nc.scalar.copy(
                                v_sb[:, hc * T + 128 * g:hc * T + 128 * (g + 1)],
                                pvs[ci][:, 128 * hc:128 * (hc + 1)],
                            )

            if tt < NTT - 1:
                pending = (block_closures(0, b_cur, jql)
                           + block_closures(1, b_cur, jql))
            else:
                # final blocks: PE-side denominator so their completion
                # (which gates the AllToAll) doesn't wait on the DVE queue
                pending = (block_closures(0, b_cur, jql, pe_denom=True)
                           + block_closures(1, b_cur, jql, pe_denom=True))

        # final block (b=1, jq=3): no projection work left to hide behind
        for f in pending:
            f()


def _phase3_outproj(nc, tc, cfg, oT_sb, woutT_d, out_d):
    T, S, HPC, DCH = cfg.T, cfg.S, cfg.HPC, cfg.DCH

    with (
        tc.tile_pool(name="dram", bufs=1, space="DRAM") as dramp,
        tc.tile_pool(name="rhsp", bufs=1) as rhsp,
        tc.tile_pool(name="woutp", bufs=2) as woutp,
        tc.tile_pool(name="osb", bufs=3) as osbp,
        tc.tile_pool(name="pout", bufs=8, space="PSUM") as poutp,
    ):
        # One A2A per head; head 0's fires while head 1's attention tail is
        # still running (collectives run off the 5 engines).
        a2a_outs = []
        for hc in range(HPC):
            ob = dramp.tile([cfg.NC * 128, cfg.TPC], F16, name=f"obounce{hc}")
            for s in range(cfg.NC):
                t0 = cfg.TPC * s
                b, sb0 = t0 // S, t0 % S
                nc.sync.dma_start(
                    ob[128 * s:128 * (s + 1), :],
                    oT_sb[:, hc * T + S * b + sb0:
                          hc * T + S * b + sb0 + cfg.TPC],
                )
            ao = dramp.tile([cfg.NC * 128, cfg.TPC], F16, name=f"a2a_out{hc}")
            nc.gpsimd.collective_compute(
                "AllToAll",
                mybir.AluOpType.bypass,
                replica_groups=[list(range(cfg.NC))],
                ins=[ob[:].opt()],
                outs=[ao[:].opt()],
            )
            a2a_outs.append(ao)

        # stream w_out per j-slice on the gpsimd queue (idle in phase 3);
        # host pre-arranged each slice contiguous -> one DMA each
        wout_js = []
        for js in range(cfg.NJS):
            wt = woutp.tile([128, DCH * 512], F16, tag="wj", name=f"wj_{js}")
            nc.gpsimd.dma_start(
                wt[:], woutT_d[:, DCH * 512 * js:DCH * 512 * (js + 1)]
            )
            wout_js.append(wt)

        # gather each feature chunk; even fc come from head 0 (early A2A)
        rhs_scr = rhsp.tile([128, DCH * cfg.TPC], F16, name="rhs_scr")
        rhs = []
        for fc in range(DCH):
            r_, hc = fc // HPC, fc % HPC
            sl = rhs_scr[:, cfg.TPC * fc:cfg.TPC * (fc + 1)]
            nc.sync.dma_start(sl, a2a_outs[hc][128 * r_:128 * (r_ + 1), :])
            rhs.append(sl)

        fcs_a = [fc for fc in range(DCH) if fc % HPC == 0]   # head 0 features
        fcs_b = [fc for fc in range(DCH) if fc % HPC != 0]   # head 1 features

        # two-pass accumulation over js-pairs: pass A (head-0 features) can
        # start as soon as the first A2A lands, hiding the second A2A
        for jp in range(cfg.NJS // 2):
            pss = {}
            for js in (2 * jp, 2 * jp + 1):
                for ti in range(cfg.NTI):
                    ps = poutp.tile([128, 512], F32, tag="po",
                                    name=f"po_{js}_{ti}")
                    pss[(js, ti)] = ps
                    for i, fc in enumerate(fcs_a):
                        nc.tensor.matmul(
                            ps[:],
                            rhs[fc][:, 128 * ti:128 * (ti + 1)],
                            wout_js[js][:, 512 * fc:512 * (fc + 1)],
                            start=(i == 0), stop=False,
                        )
            for js in (2 * jp, 2 * jp + 1):
                for ti in range(cfg.NTI):
                    ps = pss[(js, ti)]
                    for i, fc in enumerate(fcs_b):
                        nc.tensor.matmul(
                            ps[:],
                            rhs[fc][:, 128 * ti:128 * (ti + 1)],
                            wout_js[js][:, 512 * fc:512 * (fc + 1)],
                            start=False, stop=(i == len(fcs_b) - 1),
                        )
                    osb = osbp.tile([128, 512], F32, tag="ob",
                                    name=f"ob_{js}_{ti}")
                    nc.vector.tensor_copy(osb[:], ps[:])
                    nc.sync.dma_start(
                        out_d[128 * ti:128 * (ti + 1), 512 * js:512 * (js + 1)],
                        osb[:],
                    )


def _build_body(nc, tc, cfg, xT_d, wqkvT_d, cosF_d, sinF_d, masks_d, ones_d,
                woutT_d, out_d):
    T, HPC, TQ = cfg.T, cfg.HPC, cfg.TQ

    with tc.tile_pool(name="const", bufs=1) as constp:
        ones_sb = constp.tile([128, 128], F16)
        nc.gpsimd.dma_start(ones_sb[:], ones_d[:])
        masks_sb = constp.tile([128, 4 * TQ], F16)
        nc.gpsimd.dma_start(masks_sb[:], masks_d[:])
        # warm the Exp activation table off the critical path
        warm = constp.tile([128, 1], F32, name="actwarm")
        nc.scalar.activation(
            warm[:], ones_sb[:, 0:1], mybir.ActivationFunctionType.Exp
        )

        with tc.tile_pool(name="qkvp", bufs=1) as qkvp:
            q_sb = qkvp.tile([128, HPC * T], F16)
            k_sb = qkvp.tile([128, HPC * T], F16)
            v_sb = qkvp.tile([128, HPC * T], F16)
            oT_sb = qkvp.tile([128, HPC * T], F16)

            _stream(nc, tc, cfg, xT_d, wqkvT_d, cosF_d, sinF_d,
                    q_sb, k_sb, v_sb, oT_sb, masks_sb, ones_sb)
            _phase3_outproj(nc, tc, cfg, oT_sb, woutT_d, out_d)


# --------------------------------------------------------------------------
# host entry point
# --------------------------------------------------------------------------

_CACHE = {}


def _compiled(cfg):
    key = (cfg.B, cfg.S, cfg.D, cfg.H, cfg.NC)
    if key not in _CACHE:
        _CACHE[key] = build_program(cfg)
    return _CACHE[key]


def make_in_maps(cfg, inputs):
    shared, wqkvT = host_prep(
        cfg, inputs["x"], inputs["w_qkv"], inputs["w_out"],
        inputs["cos"], inputs["sin"],
    )
    return [{**shared, "wqkvT": wqkvT[c]} for c in range(cfg.NC)]


def assemble(cfg, results):
    out = np.concatenate([results[c]["out"] for c in range(cfg.NC)], axis=0)
    return out.reshape(cfg.B, cfg.S, cfg.D).astype(np.float32)


def kernel(x, w_qkv, w_out, cos, sin):
    cfg = FULL
    nc = _compiled(cfg)
    in_maps = make_in_maps(cfg, dict(x=x, w_qkv=w_qkv, w_out=w_out, cos=cos, sin=sin))
    res = bass_utils.run_bass_kernel_spmd(nc, in_maps, core_ids=list(range(cfg.NC)))
    return assemble(cfg, res.results)
